# revision 4
# baseline (speedup 1.0000x reference)
"""BiLinearInteractionLayer (bilinear_type='all') Trainium2 Bass kernel.

Contract: kernel(inputs=[2048,40,64] f32, w=[64,64] f32) -> [2048, 49920] f32,
matching

    xw  = einsum('bfd,de->bfe', inputs, w)
    p   = xw[:, I, :] * inputs[:, J, :]   # (I, J) = triu_indices(40, k=1)
    out = p.reshape(B, -1)

Data-parallel over 8 NeuronCores: batch 2048 -> 8 x 256, W replicated.
Per core, each 128-row batch tile:
  - x tile [128, 2560] DMAs to SBUF (tail fields first so the small tail
    pair-blocks can start immediately)
  - PE transposes field pairs ([128,128] -> PSUM), ACT copies to SBUF,
    PE matmuls against replicated W (one PSUM tile per matmul - two
    matmuls into one PSUM tile crash the runtime), ACT copies xw to SBUF
  - per leading field i: one DVE broadcast-multiply of xw[:, i-block]
    against x[:, j>i], then one DMA of the [128, (39-i)*64] block
    straight to its contiguous slice of the output row
The kernel is HBM-write bound (51 MB of output per core); warm it runs at
~148.5 us vs a ~138 us DMA floor.
"""

import numpy as np
from contextlib import ExitStack

import concourse.bass as bass  # noqa: F401  (registers engines)
import concourse.bacc as bacc
import concourse.tile as tile
import concourse.mybir as mybir
from concourse.bass_utils import run_bass_kernel_spmd

B = 2048
F = 40
D = 64
NCORES = 8
BS = B // NCORES                   # 256 rows per core
PAIRS = F * (F - 1) // 2           # 780
OUT_W = PAIRS * D                  # 49920
FD = F * D                         # 2560
DT = mybir.dt.float32

BLOCK_LEN = [F - 1 - i for i in range(F - 1)]
BLOCK_OFF = np.concatenate([[0], np.cumsum(BLOCK_LEN)[:-1]]).tolist()

# tail field-pairs first: their pair-blocks are small and depend only on
# the tail x chunk, so the output DMA stream starts earliest
SPLIT_F = 30
FP_ORDER = list(range(SPLIT_F // 2, F // 2)) + list(range(SPLIT_F // 2))

_CACHE = {}


def _build(bs: int):
    assert bs % 128 == 0
    ntiles = bs // 128
    nc = bacc.Bacc("TRN2", target_bir_lowering=False, debug=False)

    x_dram = nc.dram_tensor("x", [bs, F, D], DT, kind="ExternalInput").ap()
    w_dram = nc.dram_tensor("w", [D, D], DT, kind="ExternalInput").ap()
    id_dram = nc.dram_tensor("ident", [128, 128], DT, kind="ExternalInput").ap()
    out_dram = nc.dram_tensor("out", [bs, OUT_W], DT, kind="ExternalOutput").ap()

    x_flat = x_dram.rearrange("b f d -> b (f d)")
    c0 = SPLIT_F * D

    with tile.TileContext(nc) as tc, ExitStack() as ctx:
        const_pool = ctx.enter_context(tc.tile_pool(name="const", bufs=1))
        x_pool = ctx.enter_context(tc.tile_pool(name="x", bufs=2))
        xw_pool = ctx.enter_context(tc.tile_pool(name="xw", bufs=2))
        tr_pool = ctx.enter_context(tc.tile_pool(name="tr", bufs=3))
        stage_pool = ctx.enter_context(tc.tile_pool(name="stage", bufs=8))
        psum_tr = ctx.enter_context(tc.tile_pool(name="psum_tr", bufs=2, space="PSUM"))
        psum_mm = ctx.enter_context(tc.tile_pool(name="psum_mm", bufs=4, space="PSUM"))

        ident = const_pool.tile([128, 128], DT)
        nc.scalar.dma_start(ident[:], id_dram)
        # W on both partition halves so the two per-pair matmuls read lhsT
        # and rhs from the same base partition
        w_sb = const_pool.tile([128, D], DT)
        nc.scalar.dma_start(w_sb[0:D, :], w_dram)
        nc.scalar.dma_start(w_sb[D:128, :], w_dram)

        x_tiles = []
        for t in range(ntiles):
            b0 = t * 128
            x_t = x_pool.tile([128, FD], DT)
            x_tiles.append(x_t)
            # tail fields first (sync ring), rest on the scalar ring
            nc.sync.dma_start(x_t[:, c0:FD], x_flat[b0 : b0 + 128, c0:FD])
            nc.scalar.dma_start(x_t[:, 0:c0], x_flat[b0 : b0 + 128, 0:c0])

        for t in range(ntiles):
            b0 = t * 128
            x_t = x_tiles[t]
            xw_t = xw_pool.tile([128, FD], DT)
            for fp in FP_ORDER:
                tr_ps = psum_tr.tile([128, 128], DT)
                nc.tensor.transpose(
                    tr_ps[:], x_t[:, fp * 128 : (fp + 1) * 128], ident[:]
                )
                tr_sb = tr_pool.tile([128, 128], DT)
                nc.scalar.copy(tr_sb[:], tr_ps[:])
                for h in range(2):
                    i = 2 * fp + h
                    mm = psum_mm.tile([128, D], DT, tag="mm")
                    nc.tensor.matmul(
                        mm[:],
                        tr_sb[h * D : (h + 1) * D, :],
                        w_sb[h * D : (h + 1) * D, :],
                        start=True,
                        stop=True,
                    )
                    nc.scalar.copy(xw_t[:, i * D : (i + 1) * D], mm[:])
                for h in range(2):
                    i = 2 * fp + h
                    if i > F - 2:
                        continue  # field 39 never leads a pair
                    jn = F - 1 - i
                    st = stage_pool.tile([128, jn * D], DT)
                    in0 = (
                        xw_t[:, i * D : (i + 1) * D]
                        .unsqueeze(1)
                        .broadcast_to([128, jn, D])
                    )
                    in1 = x_t[:, (i + 1) * D : FD].rearrange(
                        "p (j d) -> p j d", d=D
                    )
                    nc.vector.tensor_mul(
                        st[:].rearrange("p (j d) -> p j d", d=D), in0, in1
                    )
                    nc.sync.dma_start(
                        out_dram[
                            b0 : b0 + 128,
                            BLOCK_OFF[i] * D : (BLOCK_OFF[i] + jn) * D,
                        ],
                        st[:],
                    )

    nc.compile()
    return nc


def _get_nc(bs: int):
    if bs not in _CACHE:
        _CACHE[bs] = _build(bs)
    return _CACHE[bs]


def _run(inputs: np.ndarray, w: np.ndarray, trace: bool = False):
    inputs = np.ascontiguousarray(inputs, dtype=np.float32)
    w = np.ascontiguousarray(w, dtype=np.float32)
    assert inputs.shape == (B, F, D) and w.shape == (D, D)
    nc = _get_nc(BS)
    ident = np.eye(128, dtype=np.float32)
    in_maps = [
        {"x": inputs[c * BS : (c + 1) * BS], "w": w, "ident": ident}
        for c in range(NCORES)
    ]
    res = run_bass_kernel_spmd(nc, in_maps, list(range(NCORES)), trace=trace)
    out = np.concatenate([res.results[c]["out"] for c in range(NCORES)], axis=0)
    return out, res


def kernel(inputs: np.ndarray, w: np.ndarray) -> np.ndarray:
    out, _ = _run(inputs, w)
    return out


# revision 5
# speedup vs baseline: 1.1109x; 1.1109x over previous
"""BiLinearInteractionLayer (bilinear_type='all') Trainium2 Bass kernel.

Contract: kernel(inputs=[2048,40,64] f32, w=[64,64] f32) -> [2048, 49920] f32,
matching

    xw  = einsum('bfd,de->bfe', inputs, w)
    p   = xw[:, I, :] * inputs[:, J, :]   # (I, J) = triu_indices(40, k=1)
    out = p.reshape(B, -1)

Data-parallel over 8 NeuronCores: batch 2048 -> 8 x 256, W replicated.
Per core, each 128-row batch tile:
  - x tile [128, 2560] DMAs to SBUF (tail fields first so the small tail
    pair-blocks can start immediately)
  - PE transposes field pairs ([128,128] -> PSUM), ACT copies to SBUF,
    PE matmuls against replicated W (one PSUM tile per matmul - two
    matmuls into one PSUM tile crash the runtime), ACT copies xw to SBUF
  - per leading field i: one DVE broadcast-multiply of xw[:, i-block]
    against x[:, j>i], then one DMA of the [128, (39-i)*64] block
    straight to its contiguous slice of the output row
The kernel is HBM-write bound (51 MB of output per core); warm it runs at
~148.5 us vs a ~138 us DMA floor.
"""

import numpy as np
from contextlib import ExitStack

import concourse.bass as bass  # noqa: F401  (registers engines)
import concourse.bacc as bacc
import concourse.tile as tile
import concourse.mybir as mybir
from concourse.bass_utils import run_bass_kernel_spmd

B = 2048
F = 40
D = 64
NCORES = 8
BS = B // NCORES                   # 256 rows per core
PAIRS = F * (F - 1) // 2           # 780
OUT_W = PAIRS * D                  # 49920
FD = F * D                         # 2560
DT = mybir.dt.float32

BLOCK_LEN = [F - 1 - i for i in range(F - 1)]
BLOCK_OFF = np.concatenate([[0], np.cumsum(BLOCK_LEN)[:-1]]).tolist()

# tail field-pairs first: their pair-blocks are small and depend only on
# the tail x chunk, so the output DMA stream starts earliest
SPLIT_F = 30
FP_ORDER = list(range(SPLIT_F // 2, F // 2)) + list(range(SPLIT_F // 2))

_CACHE = {}


def _build(bs: int):
    assert bs % 128 == 0
    ntiles = bs // 128
    nc = bacc.Bacc("TRN2", target_bir_lowering=False, debug=False)

    x_dram = nc.dram_tensor("x", [bs, F, D], DT, kind="ExternalInput").ap()
    w_dram = nc.dram_tensor("w", [D, D], DT, kind="ExternalInput").ap()
    id_dram = nc.dram_tensor("ident", [128, 128], DT, kind="ExternalInput").ap()
    out_dram = nc.dram_tensor("out", [bs, OUT_W], DT, kind="ExternalOutput").ap()

    x_flat = x_dram.rearrange("b f d -> b (f d)")
    c0 = SPLIT_F * D

    with tile.TileContext(nc) as tc, ExitStack() as ctx:
        const_pool = ctx.enter_context(tc.tile_pool(name="const", bufs=1))
        x_pool = ctx.enter_context(tc.tile_pool(name="x", bufs=2))
        xw_pool = ctx.enter_context(tc.tile_pool(name="xw", bufs=2))
        tr_pool = ctx.enter_context(tc.tile_pool(name="tr", bufs=3))
        stage_pool = ctx.enter_context(tc.tile_pool(name="stage", bufs=10))
        psum_tr = ctx.enter_context(tc.tile_pool(name="psum_tr", bufs=2, space="PSUM"))
        psum_mm = ctx.enter_context(tc.tile_pool(name="psum_mm", bufs=4, space="PSUM"))

        ident = const_pool.tile([128, 128], DT)
        nc.scalar.dma_start(ident[:], id_dram)
        # W on both partition halves so the two per-pair matmuls read lhsT
        # and rhs from the same base partition
        w_sb = const_pool.tile([128, D], DT)
        nc.scalar.dma_start(w_sb[0:D, :], w_dram)
        nc.scalar.dma_start(w_sb[D:128, :], w_dram)

        x_tiles = []
        for t in range(ntiles):
            b0 = t * 128
            x_t = x_pool.tile([128, FD], DT)
            x_tiles.append(x_t)
            # tail fields first (sync ring), rest on the scalar ring
            nc.sync.dma_start(x_t[:, c0:FD], x_flat[b0 : b0 + 128, c0:FD])
            nc.scalar.dma_start(x_t[:, 0:c0], x_flat[b0 : b0 + 128, 0:c0])

        for t in range(ntiles):
            b0 = t * 128
            x_t = x_tiles[t]
            xw_t = xw_pool.tile([128, FD], DT)
            for fp in FP_ORDER:
                tr_ps = psum_tr.tile([128, 128], DT)
                nc.tensor.transpose(
                    tr_ps[:], x_t[:, fp * 128 : (fp + 1) * 128], ident[:]
                )
                tr_sb = tr_pool.tile([128, 128], DT)
                nc.scalar.copy(tr_sb[:], tr_ps[:])
                for h in range(2):
                    i = 2 * fp + h
                    mm = psum_mm.tile([128, D], DT, tag="mm")
                    nc.tensor.matmul(
                        mm[:],
                        tr_sb[h * D : (h + 1) * D, :],
                        w_sb[h * D : (h + 1) * D, :],
                        start=True,
                        stop=True,
                    )
                    nc.scalar.copy(xw_t[:, i * D : (i + 1) * D], mm[:])
                for h in range(2):
                    i = 2 * fp + h
                    if i > F - 2:
                        continue  # field 39 never leads a pair
                    jn = F - 1 - i
                    st = stage_pool.tile([128, jn * D], DT)
                    in0 = (
                        xw_t[:, i * D : (i + 1) * D]
                        .unsqueeze(1)
                        .broadcast_to([128, jn, D])
                    )
                    in1 = x_t[:, (i + 1) * D : FD].rearrange(
                        "p (j d) -> p j d", d=D
                    )
                    nc.vector.tensor_mul(
                        st[:].rearrange("p (j d) -> p j d", d=D), in0, in1
                    )
                    nc.sync.dma_start(
                        out_dram[
                            b0 : b0 + 128,
                            BLOCK_OFF[i] * D : (BLOCK_OFF[i] + jn) * D,
                        ],
                        st[:],
                    )

    nc.compile()
    return nc


def _get_nc(bs: int):
    if bs not in _CACHE:
        _CACHE[bs] = _build(bs)
    return _CACHE[bs]


def _run(inputs: np.ndarray, w: np.ndarray, trace: bool = False):
    inputs = np.ascontiguousarray(inputs, dtype=np.float32)
    w = np.ascontiguousarray(w, dtype=np.float32)
    assert inputs.shape == (B, F, D) and w.shape == (D, D)
    nc = _get_nc(BS)
    ident = np.eye(128, dtype=np.float32)
    in_maps = [
        {"x": inputs[c * BS : (c + 1) * BS], "w": w, "ident": ident}
        for c in range(NCORES)
    ]
    res = run_bass_kernel_spmd(nc, in_maps, list(range(NCORES)), trace=trace)
    out = np.concatenate([res.results[c]["out"] for c in range(NCORES)], axis=0)
    return out, res


def kernel(inputs: np.ndarray, w: np.ndarray) -> np.ndarray:
    out, _ = _run(inputs, w)
    return out
